# revision 16
# baseline (speedup 1.0000x reference)
"""BiLSTM-CRF forward NLL on 8 Trainium2 NeuronCores.

Sharding: pure data-parallel over batch (8 sequences per core), params
replicated. Per core: embedding gather -> bulk input matmuls -> 2-layer
BiLSTM recurrence (fwd/bwd chains interleaved per layer) -> emissions ->
CRF forward pass -> partial (em_sel, denom) pair. Host sums partials
with the label-dependent numerator constant.

Schedule: the recurrence dependency chain (~1.7us per time step, 1024
sequential steps) is the critical path; everything else is hidden inside
its engine-idle time:
  * embedding gather + layer-0 input matmuls stream chunk-wise from both
    sequence ends so the L0 recurrence starts after ~2 chunks, not after
    the full batch;
  * layer-1 input matmuls run inside L0's later slots as soon as both
    directions' h outputs for a time chunk exist;
  * emissions + exp + CRF chunk-matrix products run inside L1's slots.
    The CRF forward scan is reassociated into per-chunk 9x9 matrix
    products P -> (prod_t diag(em_t) E^T) P, which are independent
    across (chunk, sequence) and therefore batchable; only a short
    per-chunk combine remains at the end.

LSTM cell math (validated exactly against the reference in fp32):
  * single tanh activation per step over all 4 gates:
    sigmoid(x) = (tanh(x/2)+1)/2, with the tanh(0.5*...) instruction scale
    folded into host-prescaled weights (g-gate rows x2). The cell tracks
    C = 2c and H = 2h; every consumer of h (recurrent weights, layer-1
    input weights, output projection) is pre-halved on the host.
  * CRF runs in probability space: em' = em + b_out - log(L); the
    per-step shift log(L) cancels exactly between numerator and
    denominator. Combine renormalizes r every few chunks.
"""

import os
import sys

import numpy as np

sys.path.insert(0, "/opt/trn_rl_repo")

import concourse.bass as bass
import concourse.tile as tile
from concourse import bacc, mybir
from concourse.bass_utils import run_bass_kernel_spmd

B, T, V, D, HD, L = 64, 512, 100000, 300, 256, 9
H = 128
NCORES = 8
BL = B // NCORES          # sequences per core
DPAD = 384                # D padded so DMA-transpose chunks are 128 wide
KCH = (128, 128, 128)     # K chunks of DPAD
CBAR = float(np.log(L))   # per-step CRF shift (cancels in num - denom)

f32 = mybir.dt.float32
bf16 = mybir.dt.bfloat16
i32 = mybir.dt.int32
ALU = mybir.AluOpType
ACTF = mybir.ActivationFunctionType


def _crf_chunks(Tsteps):
    """(t0, t1) chunk list; chunk covers transition matrices t in [t0,t1).
    Small chunks at both sequence ends (their inputs become available only
    at the very end of the L1 recurrence), big chunks in the middle."""
    if Tsteps < 64:
        return [(1, Tsteps)]
    chunks = [(1, 8), (8, 16), (16, 24), (24, 32)]
    c = 32
    while c < Tsteps - 32:
        chunks.append((c, c + 32))
        c += 32
    chunks += [(c, c + 8), (c + 8, c + 16), (c + 16, c + 24),
               (c + 24, c + 32)]
    return chunks


# ---------------------------------------------------------------------------
# device program
# ---------------------------------------------------------------------------

def build_program(Tsteps=T):
    NT = Tsteps * BL
    NTILES = NT // 128                 # gather tiles (128 tokens each)
    assert NT % 128 == 0

    nc = bacc.Bacc("TRN2", target_bir_lowering=False, debug=False)

    def din(name, shape, dt):
        return nc.dram_tensor(name, shape, dt, kind="ExternalInput").ap()

    words = din("words", [NTILES, 128, 1], i32)
    emb = din("emb", [V, DPAD], bf16)
    ident = din("ident", [128, 128], bf16)
    # lhsT weights, gate-major free dim (slots i,f,g,o each 128 wide)
    wih0 = din("wih0", [2, 3, 128, 512], bf16)     # [dir][kchunk][K][4*128]
    wih1 = din("wih1", [2, 2, 128, 512], bf16)     # [dir][h0-dir kchunk][K][4*128]
    whh = din("whh", [2, 2, 128, 512], bf16)       # [layer][dir][K=H][4*128]
    biases = din("biases", [2, 2, 128, 4], f32)    # [layer][dir][hidden][gate]
    woutT = din("woutT", [2, 128, L], bf16)        # [h1-dir kchunk][K][L]
    bout = din("bout", [L, 1], f32)                # b_out - CBAR
    startv = din("startv", [L, 1], f32)
    expEb = din("expEb", [L, L], bf16)             # exp(trans), matmul lhsT
    expET = din("expET", [L, L], f32)              # exp(trans)^T
    expend = din("expend", [L, 1], f32)            # exp(end_t)
    oh = din("oh", [L, NT], bf16)                  # label one-hot, (t,b) order
    res = nc.dram_tensor("res", [1, 2], f32, kind="ExternalOutput").ap()

    with tile.TileContext(nc) as tc:
        _emit(tc, nc, Tsteps, NT, NTILES,
              words, emb, ident, wih0, wih1, whh, biases, woutT, bout,
              startv, expEb, expET, expend, oh, res)
    nc.compile()
    return nc


def _emit(tc, nc, Tsteps, NT, NTILES,
          words, emb, ident, wih0, wih1, whh, biases, woutT, bout,
          startv, expEb, expET, expend, oh, res):
    from contextlib import ExitStack

    NCH = max(1, Tsteps // 32)         # bulk chunks
    TCH = Tsteps // NCH                # time steps per bulk chunk
    CCOLS = TCH * BL                   # xp columns per bulk chunk
    chunks = _crf_chunks(Tsteps)
    NCRF = len(chunks)

    ctx = ExitStack()
    with ctx:
        consts = ctx.enter_context(tc.tile_pool(name="consts", bufs=1))
        states = ctx.enter_context(tc.tile_pool(name="states", bufs=1))

        # ---- persistent SBUF tiles ----
        ident_sb = consts.tile([128, 128], bf16, tag="ident")
        nc.sync.dma_start(ident_sb[:], ident[:])
        whh_sb = {}
        for l in range(2):
            for d in range(2):
                t_ = consts.tile([128, 512], bf16, name=f"whh{l}{d}")
                nc.sync.dma_start(t_[:], whh[l, d])
                whh_sb[l, d] = t_
        bias_sb = {}
        for l in range(2):
            for d in range(2):
                t_ = consts.tile([128, 4], f32, name=f"bias{l}{d}")
                nc.sync.dma_start(t_[:], biases[l, d])
                bias_sb[l, d] = t_
        wih0_sb = {}
        for d in range(2):
            for c in range(3):
                t_ = consts.tile([128, 512], bf16, name=f"wih0_{d}{c}")
                nc.sync.dma_start(t_[:], wih0[d, c])
                wih0_sb[d, c] = t_
        wih1_sb = {}
        for d in range(2):
            for k in range(2):
                t_ = consts.tile([128, 512], bf16, name=f"wih1_{d}{k}")
                nc.sync.dma_start(t_[:], wih1[d, k])
                wih1_sb[d, k] = t_
        woutT_sb = [consts.tile([128, L], bf16, name=f"wo{k}") for k in range(2)]
        for k in range(2):
            nc.sync.dma_start(woutT_sb[k][:], woutT[k])
        bout_sb = consts.tile([L, 1], f32, name="bout_sb")
        start_sb = consts.tile([L, 1], f32, name="start_sb")
        expEb_sb = consts.tile([L, L], bf16, name="expEb_sb")
        expET_sb = consts.tile([L, L], f32, name="expET_sb")
        expend_sb = consts.tile([L, 1], f32, name="expend_sb")
        nc.sync.dma_start(bout_sb[:], bout[:])
        nc.sync.dma_start(start_sb[:], startv[:])
        nc.sync.dma_start(expEb_sb[:], expEb[:])
        nc.sync.dma_start(expET_sb[:], expET[:])
        nc.sync.dma_start(expend_sb[:], expend[:])
        ones91 = consts.tile([L, 1], f32, name="ones91")
        ones91b = consts.tile([L, 1], bf16, name="ones91b")
        ones19 = consts.tile([1, L], f32, name="ones19")
        ones98 = consts.tile([L, BL], f32, name="ones98")
        nc.vector.memset(ones91[:], 1.0)
        nc.vector.memset(ones91b[:], 1.0)
        nc.vector.memset(ones19[:], 1.0)
        nc.vector.memset(ones98[:], 1.0)

        h_hist = {}
        for l in range(2):
            for d in range(2):
                h_hist[l, d] = states.tile([128, NT], bf16, name=f"h{l}{d}")
        C_st = {}
        for l in range(2):
            for d in range(2):
                C_st[l, d] = states.tile([128, BL], f32, name=f"C{l}{d}")

        # xp for the two directions of the current layer (reused across layers)
        xp_sb = [states.tile([128, 4 * NT], bf16, name=f"xp{d}") for d in range(2)]

        # emissions / CRF state
        em_sb = states.tile([L, NT], bf16, name="em")
        expem = states.tile([L, NT], f32, name="expem")
        oh_sb = states.tile([L, NT], bf16, name="oh_sb")
        nc.sync.dma_start(oh_sb[:], oh[:])
        num_parts = states.tile([L, NCRF], f32, name="num_parts")
        lnacc = states.tile([1, BL], f32, name="lnacc")
        r_sb = states.tile([L, BL], bf16, name="r_sb")
        nc.vector.memset(lnacc[:], 0.0)
        # X chunk-product tiles, j-major cols (col = j*BL + b)
        X_sb = [states.tile([L, L * BL], bf16, name=f"X{ci}")
                for ci in range(NCRF)]

        # =================================================================
        # embedding gather: all tiles up-front, interleaved from both ends
        # so early-needed chunks land first. DMA queues stream ahead of
        # compute.
        # =================================================================
        x_T = [states.tile([k, NT], bf16, name=f"xT{c}") for c, k in enumerate(KCH)]
        gorder = []
        lo, hi = 0, NTILES - 1
        while lo <= hi:
            gorder.append(lo)
            if hi != lo:
                gorder.append(hi)
            lo += 1
            hi -= 1
        gp_pool = ctx.enter_context(tc.tile_pool(name="gath", bufs=4))
        ip_pool = ctx.enter_context(tc.tile_pool(name="idx", bufs=4))
        for i in gorder:
            idx = ip_pool.tile([128, 1], i32, tag="idx", name="idx")
            nc.sync.dma_start(idx[:], words[i])
            g = gp_pool.tile([128, DPAD], bf16, tag="g", name="g")
            nc.gpsimd.indirect_dma_start(
                out=g[:], out_offset=None, in_=emb[:],
                in_offset=bass.IndirectOffsetOnAxis(ap=idx[:, :1], axis=0),
            )
            off = 0
            for c, k in enumerate(KCH):
                nc.sync.dma_start_transpose(
                    x_T[c][:, i * 128:(i + 1) * 128], g[:, off:off + k])
                off += k

        # =================================================================
        # bulk-matmul thunks (layer-0 input projection, layer-1 input
        # projection). One thunk = one (dir, gate-slot) chunk: K-accumulated
        # matmuls into PSUM + bias add into xp.
        # =================================================================
        b0pt = ctx.enter_context(tc.tile_pool(name="b0pt", bufs=2, space="PSUM"))

        def bulk_thunk(layer, d, slot, ch, pool):
            srcs = ([(x_T[c], wih0_sb[d, c], KCH[c]) for c in range(3)]
                    if layer == 0 else
                    [(h_hist[0, k], wih1_sb[d, k], 128) for k in range(2)])
            nsl = slice(ch * CCOLS, (ch + 1) * CCOLS)

            def fn():
                pt = pool.tile([128, CCOLS], f32, tag="pt", name="pt")
                for ki, (src, wt, kk) in enumerate(srcs):
                    nc.tensor.matmul(
                        pt[:],
                        lhsT=wt[:kk, slot * 128:(slot + 1) * 128],
                        rhs=src[:kk, nsl],
                        start=(ki == 0), stop=(ki == len(srcs) - 1),
                    )
                xv = xp_sb[d][:].rearrange("p (t g b) -> p t g b", g=4, b=BL)
                nc.vector.tensor_scalar(
                    out=xv[:, ch * TCH:(ch + 1) * TCH, slot, :],
                    in0=pt[:].rearrange("p (t b) -> p t b", b=BL),
                    scalar1=bias_sb[layer, d][:, slot:slot + 1],
                    scalar2=None, op0=ALU.add,
                )
            return fn

        def bulk_chunk_thunks(layer, ch, pool):
            return [bulk_thunk(layer, d, slot, ch, pool)
                    for d in range(2) for slot in range(4)]

        # =================================================================
        # CRF chunk thunks: emissions matmul + bias, exp, numerator
        # reduce, chunk-product init + steps.
        # =================================================================
        empt = ctx.enter_context(tc.tile_pool(name="empt", bufs=1, space="PSUM"))
        qtmp = ctx.enter_context(tc.tile_pool(name="qtmp", bufs=2, space="PSUM"))
        scrp = ctx.enter_context(tc.tile_pool(name="scrp", bufs=2))

        def crf_chunk_thunks(ci, t0, t1):
            c0 = (t0 - 1) * BL if t0 == 1 else t0 * BL   # chunk 0 also covers t=0
            c1 = t1 * BL
            X = X_sb[ci]
            Xr3 = X[:, 0:L * BL].rearrange("p (j b) -> p j b", b=BL)
            thunks = []

            w = c1 - c0
            hold_em = {}

            def em_mms():
                pt = empt.tile([L, CCOLS], f32, tag="empt", name="pt")
                for k in range(2):
                    nc.tensor.matmul(pt[:, 0:w], lhsT=woutT_sb[k][:],
                                     rhs=h_hist[1, k][:, c0:c1],
                                     start=(k == 0), stop=(k == 1))
                hold_em["pt"] = pt
            thunks.append(em_mms)

            def em_bias():
                nc.vector.tensor_scalar(
                    out=em_sb[:, c0:c1], in0=hold_em["pt"][:, 0:w],
                    scalar1=bout_sb[:, 0:1], scalar2=None, op0=ALU.add)
            thunks.append(em_bias)

            def em_exp():
                nc.scalar.activation(expem[:, c0:c1], em_sb[:, c0:c1], ACTF.Exp)
            thunks.append(em_exp)

            def em_num():
                scr = scrp.tile([L, c1 - c0], f32, tag=f"scr{c1 - c0}", name="scr")
                nc.vector.tensor_tensor(out=scr[:], in0=em_sb[:, c0:c1],
                                        in1=oh_sb[:, c0:c1], op=ALU.mult)
                nc.vector.tensor_reduce(num_parts[:, ci:ci + 1], scr[:],
                                        axis=mybir.AxisListType.X, op=ALU.add)
            thunks.append(em_num)

            def x_init():
                emB = expem[:, t0 * BL:(t0 + 1) * BL].unsqueeze(1) \
                    .broadcast_to([L, L, BL])
                ETB = expET_sb[:].unsqueeze(2).broadcast_to([L, L, BL])
                nc.vector.tensor_tensor(out=Xr3, in0=ETB, in1=emB, op=ALU.mult)
            thunks.append(x_init)

            for t in range(t0 + 1, t1):
                hold = {}

                def q_mm(t=t, hold=hold):
                    tmp = qtmp.tile([L, L * BL], f32, tag="tmp", name="tmp")
                    nc.tensor.matmul(tmp[:], lhsT=expEb_sb[:], rhs=X[:, 0:L * BL],
                                     start=True, stop=True)
                    hold["tmp"] = tmp

                def q_tt(t=t, hold=hold):
                    emB = expem[:, t * BL:(t + 1) * BL].unsqueeze(1) \
                        .broadcast_to([L, L, BL])
                    nc.vector.tensor_tensor(
                        out=Xr3,
                        in0=hold["tmp"][:].rearrange("p (j b) -> p j b", b=BL),
                        in1=emB, op=ALU.mult)
                thunks.append(q_mm)
                thunks.append(q_tt)
            return thunks

        # =================================================================
        # LSTM recurrence with side-work calendar hooks
        # =================================================================
        def recur_phase(layer, cal_pre, cal_post):
            for d in range(2):
                nc.vector.memset(C_st[layer, d][:], 0.0)
            with tc.tile_pool(name=f"gates{layer}", bufs=3, space="PSUM") as gpp, \
                 tc.tile_pool(name=f"y{layer}", bufs=4) as yp, \
                 tc.tile_pool(name=f"v{layer}", bufs=6) as vp:
                for t in range(Tsteps):
                    for fn in cal_pre.get(t, ()):
                        fn()
                    taus = (t, Tsteps - 1 - t)
                    first = (t == 0)
                    ys, tcs = {}, {}
                    gpt = gpp.tile([128, 8 * BL], f32, tag="gp", name="gp")
                    gps = {d: gpt[:, d * 4 * BL:(d + 1) * 4 * BL] for d in range(2)}
                    for d in range(2):
                        tau = taus[d]
                        gp = gps[d]
                        nc.tensor.matmul(gp, lhsT=ident_sb[:],
                                         rhs=xp_sb[d][:, tau * 4 * BL:(tau + 1) * 4 * BL],
                                         start=True, stop=first)
                        if not first:
                            prev = tau - 1 if d == 0 else tau + 1
                            hh = h_hist[layer, d]
                            whh_t = whh_sb[layer, d]
                            for slot in range(4):
                                nc.tensor.matmul(
                                    gp[:, slot * BL:(slot + 1) * BL],
                                    lhsT=whh_t[:, slot * 128:(slot + 1) * 128],
                                    rhs=hh[:, prev * BL:(prev + 1) * BL],
                                    start=False, stop=(slot == 3))
                    for d in range(2):
                        y = yp.tile([128, 4 * BL], f32, tag=f"y{d}", name=f"y{d}")
                        ys[d] = y
                        nc.scalar.activation(y[:], gps[d], ACTF.Tanh, scale=0.5)
                    for d in range(2):
                        y, C = ys[d], C_st[layer, d]
                        v2 = vp.tile([128, BL], f32, tag=f"v2{d}", name=f"v2{d}")
                        z = vp.tile([128, BL], f32, tag=f"z{d}", name=f"z{d}")
                        nc.vector.scalar_tensor_tensor(
                            v2[:], in0=y[:, 0:BL], scalar=1.0, in1=y[:, 2 * BL:3 * BL],
                            op0=ALU.add, op1=ALU.mult)
                        nc.vector.scalar_tensor_tensor(
                            z[:], in0=y[:, BL:2 * BL], scalar=1.0, in1=C[:],
                            op0=ALU.add, op1=ALU.mult)
                        nc.vector.scalar_tensor_tensor(
                            C[:], in0=z[:], scalar=0.5, in1=v2[:],
                            op0=ALU.mult, op1=ALU.add)
                    for d in range(2):
                        tcl = vp.tile([128, BL], f32, tag=f"tc{d}", name=f"tc{d}")
                        tcs[d] = tcl
                        nc.scalar.activation(tcl[:], C_st[layer, d][:], ACTF.Tanh,
                                             scale=0.5)
                    for d in range(2):
                        tau = taus[d]
                        nc.vector.scalar_tensor_tensor(
                            h_hist[layer, d][:, tau * BL:(tau + 1) * BL],
                            in0=ys[d][:, 3 * BL:4 * BL], scalar=1.0, in1=tcs[d][:],
                            op0=ALU.add, op1=ALU.mult)
                    for fn in cal_post.get(t, ()):
                        fn()

        # calendar helper: place thunks one per half-slot (cap per half)
        def place(cal_pre, cal_post, thunks, start, cap=1):
            leftover = []
            s, h, c = start, 0, 0
            for fn in thunks:
                if s >= Tsteps:
                    leftover.append(fn)
                    continue
                (cal_pre if h == 0 else cal_post).setdefault(s, []).append(fn)
                c += 1
                if c >= cap:
                    c = 0
                    h += 1
                    if h == 2:
                        h = 0
                        s += 1
            return leftover

        # ---- L0 calendar: bulk0 chunk pairs streamed from both ends;
        #      bulk1 chunk pairs as soon as both dirs' h are final.
        cal0_pre, cal0_post = {}, {}
        prologue, gap_thunks = [], []
        if NCH >= 2:
            prologue += bulk_chunk_thunks(0, 0, b0pt)
            prologue += bulk_chunk_thunks(0, NCH - 1, b0pt)
            for j in range(1, (NCH + 1) // 2):
                th = bulk_chunk_thunks(0, j, b0pt)
                if NCH - 1 - j != j:
                    th += bulk_chunk_thunks(0, NCH - 1 - j, b0pt)
                place(cal0_pre, cal0_post, th, TCH * (j - 1))
            for c in range(1, NCH // 2):
                th = bulk_chunk_thunks(1, c, b0pt)
                if NCH - 1 - c != c:
                    th += bulk_chunk_thunks(1, NCH - 1 - c, b0pt)
                # h0b[TCH*c] lands during slot (Tsteps-1) - TCH*c; emit after
                place(cal0_pre, cal0_post, th, Tsteps - TCH * c)
            gap_thunks += bulk_chunk_thunks(1, 0, b0pt)
            gap_thunks += bulk_chunk_thunks(1, NCH - 1, b0pt)
            if NCH >= 2 and NCH // 2 <= NCH - 1 - (NCH // 2):
                # middle chunk (odd NCH) or none
                for c in range(NCH // 2, NCH - 1 - (NCH // 2) + 1):
                    place(cal0_pre, cal0_post, bulk_chunk_thunks(1, c, b0pt),
                          Tsteps - TCH * min(c, NCH - 1 - c))
        else:
            prologue += bulk_chunk_thunks(0, 0, b0pt)
            gap_thunks += bulk_chunk_thunks(1, 0, b0pt)

        # ---- L1 calendar: emissions + CRF chunk products.
        cal1_pre, cal1_post = {}, {}
        tails = []
        for ci, (t0, t1) in enumerate(chunks):
            tlo = t0 - 1 if t0 == 1 else t0   # em range also covers t=0
            a = max(t1 - 1, (Tsteps - 1) - tlo) + 1
            th = crf_chunk_thunks(ci, t0, t1)
            cap = 2 if a >= Tsteps - 72 else 1
            if a >= Tsteps:
                tails.append(th)
            else:
                left = place(cal1_pre, cal1_post, th, a, cap=cap)
                if left:
                    tails.append(left)

        # =================================================================
        # run it
        # =================================================================
        for fn in prologue:
            fn()
        recur_phase(0, cal0_pre, cal0_post)
        for fn in gap_thunks:
            fn()
        recur_phase(1, cal1_pre, cal1_post)

        # tail: leftover CRF chunks, round-robin interleaved for overlap
        maxlen = max((len(t) for t in tails), default=0)
        for k in range(maxlen):
            for tl in tails:
                if k < len(tl):
                    tl[k]()

        # =================================================================
        # CRF combine: r <- normalize(X_c^T r), descending over chunks.
        # Row L of each combine matmul output = sum of the incoming r
        # (ones column readout) used for the periodic renormalization.
        # =================================================================
        with tc.tile_pool(name="cmb", bufs=3, space="PSUM") as cmb, \
             tc.tile_pool(name="cmbs", bufs=4) as cmbs:
            # r init = expend (per-partition scalar mult of ones)
            nc.vector.tensor_scalar(out=r_sb[:], in0=ones98[:],
                                    scalar1=expend_sb[:, 0:1], scalar2=None,
                                    op0=ALU.mult)
            order = sorted(range(NCRF), key=lambda ci: -chunks[ci][0])
            for k, ci in enumerate(order):
                X = X_sb[ci]
                Xc = X[:].rearrange("p (j b) -> p b j", b=BL)  # [L, BL, L]
                renorm = (k % 3 == 2)
                if renorm:
                    # s = sum(r) before the update; lnacc += ln s; r /= s
                    sps = cmb.tile([L, BL], f32, tag="cps", name="sps")
                    nc.tensor.matmul(sps[0:1, :], lhsT=ones91b[:], rhs=r_sb[:],
                                     start=True, stop=True)
                rps = cmb.tile([L, BL], f32, tag="cps", name="rps")
                for b in range(BL):
                    nc.tensor.matmul(rps[:, b:b + 1], lhsT=Xc[:, b],
                                     rhs=r_sb[:, b:b + 1], start=True, stop=True)
                if renorm:
                    lns = cmbs.tile([1, BL], f32, tag="lns", name="lns")
                    nc.scalar.activation(lns[:], sps[0:1, :], ACTF.Ln)
                    nc.vector.tensor_tensor(out=lnacc[:], in0=lnacc[:],
                                            in1=lns[:], op=ALU.add)
                    rec = cmbs.tile([1, BL], f32, tag="rec", name="rec")
                    nc.vector.reciprocal(rec[:], sps[0:1, :])
                    rbt = cmb.tile([L, BL], f32, tag="cps", name="rbt")
                    nc.tensor.matmul(rbt[:], lhsT=ones19[:], rhs=rec[:],
                                     start=True, stop=True)
                    rtmp = cmbs.tile([L, BL], f32, tag="rtmp", name="rtmp")
                    nc.vector.tensor_scalar(out=rtmp[:], in0=rps[:],
                                            scalar1=1.0, scalar2=None,
                                            op0=ALU.mult)
                    nc.vector.tensor_tensor(out=r_sb[:], in0=rtmp[:],
                                            in1=rbt[:], op=ALU.mult)
                else:
                    nc.vector.tensor_scalar(out=r_sb[:], in0=rps[:],
                                            scalar1=1.0, scalar2=None,
                                            op0=ALU.mult)

            # finalize: P0 = exp(em_0 + start); z = r . P0; denom = ln z + lnacc
            P0 = cmbs.tile([L, BL], f32, tag="P0", name="P0")
            nc.scalar.activation(P0[:], em_sb[:, 0:BL], ACTF.Exp,
                                 bias=start_sb[:, 0:1])
            prod = cmbs.tile([L, BL], f32, tag="prod", name="prod")
            nc.vector.tensor_tensor(out=prod[:], in0=P0[:], in1=r_sb[:],
                                    op=ALU.mult)
            zps = cmb.tile([L, BL], f32, tag="cps", name="zps")
            nc.tensor.matmul(zps[0:1, :], lhsT=ones91[:], rhs=prod[:],
                             start=True, stop=True)
            lnz = cmbs.tile([1, BL], f32, tag="lnz", name="lnz")
            nc.scalar.activation(lnz[:], zps[0:1, :], ACTF.Ln)
            nc.vector.tensor_tensor(out=lnz[:], in0=lnz[:], in1=lnacc[:],
                                    op=ALU.add)
            dsc = cmbs.tile([1, 1], f32, tag="dsc", name="dsc")
            nc.vector.tensor_reduce(dsc[:], lnz[:], axis=mybir.AxisListType.X,
                                    op=ALU.add)
            num9 = cmbs.tile([L, 1], f32, tag="num9", name="num9")
            nc.vector.tensor_reduce(num9[:], num_parts[:],
                                    axis=mybir.AxisListType.X, op=ALU.add)
            nps = cmb.tile([L, BL], f32, tag="cps", name="nps")
            nc.tensor.matmul(nps[0:1, 0:1], lhsT=ones91[:], rhs=num9[:, 0:1],
                             start=True, stop=True)
            out_sb = cmbs.tile([1, 2], f32, tag="out_sb", name="out_sb")
            nc.vector.tensor_scalar(out=out_sb[:, 0:1], in0=nps[0:1, 0:1],
                                    scalar1=0.0, scalar2=None, op0=ALU.add)
            nc.vector.tensor_scalar(out=out_sb[:, 1:2], in0=dsc[:],
                                    scalar1=0.0, scalar2=None, op0=ALU.add)
            nc.sync.dma_start(res[:], out_sb[:])


# ---------------------------------------------------------------------------
# host side
# ---------------------------------------------------------------------------

def _prescale(w_ih, w_hh, b_ih, b_hh, h_in_doubled):
    """Gate-slot layout is torch order (i,f,g,o). Returns fp32 arrays."""
    sg = np.ones((4, 1), np.float32)
    sg[2] = 2.0                       # g gate rows x2 (tanh scale 0.5 trick)
    srows = np.repeat(sg, H, axis=0)  # [512, 1]
    wih = w_ih.astype(np.float32) * srows
    whh_ = w_hh.astype(np.float32) * srows * 0.5
    b = (b_ih + b_hh).astype(np.float32) * srows[:, 0]
    if h_in_doubled:
        wih = wih * 0.5
    return wih, whh_, b


def _lhsT_gate_major(w, kchunks):
    """w: [4H, K] fp32 -> [nchunk, 128, 512] bf16 lhsT (zero-padded K)."""
    outs = []
    off = 0
    for kk in kchunks:
        blk = np.zeros((128, 512), np.float32)
        take = min(kk, w.shape[1] - off)
        for slot in range(4):
            blk[:take, slot * 128:(slot + 1) * 128] = \
                w[slot * H:(slot + 1) * H, off:off + take].T
        outs.append(blk)
        off += kk
    return np.stack(outs).astype(np.dtype("bfloat16"))


_PROG_CACHE = {}


def _get_program(Tsteps):
    if Tsteps not in _PROG_CACHE:
        _PROG_CACHE[Tsteps] = build_program(Tsteps)
    return _PROG_CACHE[Tsteps]


def prepare_inputs(inputs, Tsteps=T):
    """Build the per-core input maps + the host numerator constants."""
    bfl = np.dtype("bfloat16")
    words = np.asarray(inputs["word_batch"]).astype(np.int64)
    labels = np.asarray(inputs["label_batch"]).astype(np.int64)
    emb = np.asarray(inputs["emb"], np.float32)
    words = words[:, :Tsteps]
    labels = labels[:, :Tsteps]

    embp = np.zeros((V, DPAD), np.float32)
    embp[:, :D] = emb
    embp = embp.astype(bfl)

    ident = np.eye(128, dtype=np.float32).astype(bfl)

    wih0_l, whh_l, wih1_l, bias_l = [], [], [], []
    for layer, (wihk, whhk, bihk, bhhk) in enumerate(
            [("w_ih_l0", "w_hh_l0", "b_ih_l0", "b_hh_l0"),
             ("w_ih_l1", "w_hh_l1", "b_ih_l1", "b_hh_l1")]):
        for d in range(2):
            wih, whh_, b = _prescale(
                np.asarray(inputs[wihk])[d], np.asarray(inputs[whhk])[d],
                np.asarray(inputs[bihk])[d], np.asarray(inputs[bhhk])[d],
                h_in_doubled=(layer == 1))
            if layer == 0:
                wihp = np.zeros((512, DPAD), np.float32)
                wihp[:, :D] = wih
                wih0_l.append(_lhsT_gate_major(wihp, KCH))
            else:
                wih1_l.append(_lhsT_gate_major(wih, (128, 128)))
            whh_l.append(_lhsT_gate_major(whh_, (128,)))
            bias_l.append(b.reshape(4, H).T)  # [128, 4] gate-major cols
    wih0 = np.stack(wih0_l)                       # [2, 3, 128, 512]
    wih1 = np.stack(wih1_l)                       # [2, 2, 128, 512]
    whh = np.stack(whh_l).reshape(2, 2, 1, 128, 512)[:, :, 0]
    biases = np.stack(bias_l).reshape(2, 2, 128, 4).astype(np.float32)

    w_out = np.asarray(inputs["w_out"], np.float32) * 0.5   # [L, 2H]
    woutT = np.stack([w_out[:, :H].T, w_out[:, H:].T]).astype(bfl)  # [2,128,L]
    bout = (np.asarray(inputs["b_out"], np.float32) - CBAR).reshape(L, 1)
    startv = np.asarray(inputs["start_t"], np.float32).reshape(L, 1)
    expE = np.exp(np.asarray(inputs["trans"], np.float32))
    expEb = expE.astype(bfl)
    expET = expE.T.copy().astype(np.float32)
    expend = np.exp(np.asarray(inputs["end_t"], np.float32)).reshape(L, 1)

    NT = Tsteps * BL
    in_maps = []
    num_consts = []
    start_t = np.asarray(inputs["start_t"], np.float32)
    end_t = np.asarray(inputs["end_t"], np.float32)
    trans = np.asarray(inputs["trans"], np.float32)
    for c in range(NCORES):
        bs = slice(c * BL, (c + 1) * BL)
        wc = words[bs]                        # [BL, Tsteps]
        lc = labels[bs]
        toks = wc.T.reshape(-1).astype(np.int32)          # (t, b) order
        ohc = (lc.T.reshape(1, -1) == np.arange(L).reshape(L, 1))
        in_maps.append({
            "words": toks.reshape(NT // 128, 128, 1),
            "emb": embp, "ident": ident,
            "wih0": wih0, "wih1": wih1, "whh": whh, "biases": biases,
            "woutT": woutT, "bout": bout, "startv": startv,
            "expEb": expEb, "expET": expET, "expend": expend,
            "oh": ohc.astype(bfl),
        })
        num_consts.append(
            float(start_t[lc[:, 0]].sum())
            + float(trans[lc[:, :-1], lc[:, 1:]].sum())
            + float(end_t[lc[:, -1]].sum()))
    return in_maps, num_consts


def kernel(**inputs):
    in_maps, num_consts = prepare_inputs(inputs, T)
    nc = _get_program(T)
    out = run_bass_kernel_spmd(nc, in_maps, list(range(NCORES)))
    llh = 0.0
    for c in range(NCORES):
        r = out.results[c]["res"].reshape(2).astype(np.float64)
        llh += num_consts[c] + r[0] - r[1]
    return np.float32(-llh)


if __name__ == "__main__":
    np.random.seed(0)
    print("building program ...")
    build_program(T)
    print("ok")


# revision 17
# speedup vs baseline: 1.1657x; 1.1657x over previous
"""BiLSTM-CRF forward NLL on 8 Trainium2 NeuronCores.

Sharding: pure data-parallel over batch (8 sequences per core), params
replicated. Per core: embedding gather -> bulk input matmuls -> 2-layer
BiLSTM recurrence (fwd/bwd chains interleaved per layer) -> emissions ->
CRF forward pass -> partial (em_sel, denom) pair. Host sums partials
with the label-dependent numerator constant.

Schedule: the recurrence dependency chain (~1.7us per time step, 1024
sequential steps) is the critical path; everything else is hidden inside
its engine-idle time:
  * embedding gather + layer-0 input matmuls stream chunk-wise from both
    sequence ends so the L0 recurrence starts after ~2 chunks, not after
    the full batch;
  * layer-1 input matmuls run inside L0's later slots as soon as both
    directions' h outputs for a time chunk exist;
  * emissions + exp + CRF chunk-matrix products run inside L1's slots.
    The CRF forward scan is reassociated into per-chunk 9x9 matrix
    products P -> (prod_t diag(em_t) E^T) P, which are independent
    across (chunk, sequence) and therefore batchable; only a short
    per-chunk combine remains at the end.

LSTM cell math (validated exactly against the reference in fp32):
  * single tanh activation per step over all 4 gates:
    sigmoid(x) = (tanh(x/2)+1)/2, with the tanh(0.5*...) instruction scale
    folded into host-prescaled weights (g-gate rows x2). The cell tracks
    C = 2c and H = 2h; every consumer of h (recurrent weights, layer-1
    input weights, output projection) is pre-halved on the host.
  * CRF runs in probability space: em' = em + b_out - log(L); the
    per-step shift log(L) cancels exactly between numerator and
    denominator. Combine renormalizes r every few chunks.
"""

import os
import sys

import numpy as np

sys.path.insert(0, "/opt/trn_rl_repo")

import concourse.bass as bass
import concourse.tile as tile
from concourse import bacc, mybir
from concourse.bass_utils import run_bass_kernel_spmd

B, T, V, D, HD, L = 64, 512, 100000, 300, 256, 9
H = 128
NCORES = 8
BL = B // NCORES          # sequences per core
DPAD = 384                # D padded so DMA-transpose chunks are 128 wide
KCH = (128, 128, 128)     # K chunks of DPAD
CBAR = float(np.log(L))   # per-step CRF shift (cancels in num - denom)

f32 = mybir.dt.float32
bf16 = mybir.dt.bfloat16
i32 = mybir.dt.int32
ALU = mybir.AluOpType
ACTF = mybir.ActivationFunctionType


def _crf_chunks(Tsteps):
    """(t0, t1) chunk list; chunk covers transition matrices t in [t0,t1).
    Small chunks at both sequence ends (their inputs become available only
    at the very end of the L1 recurrence), big chunks in the middle."""
    if Tsteps < 64:
        return [(1, Tsteps)]
    chunks = [(1, 8), (8, 16), (16, 24), (24, 32)]
    c = 32
    while c < Tsteps - 32:
        chunks.append((c, c + 32))
        c += 32
    chunks += [(c, c + 8), (c + 8, c + 16), (c + 16, c + 24),
               (c + 24, c + 32)]
    return chunks


# ---------------------------------------------------------------------------
# device program
# ---------------------------------------------------------------------------

def build_program(Tsteps=T):
    NT = Tsteps * BL
    NTILES = NT // 128                 # gather tiles (128 tokens each)
    assert NT % 128 == 0

    nc = bacc.Bacc("TRN2", target_bir_lowering=False, debug=False)

    def din(name, shape, dt):
        return nc.dram_tensor(name, shape, dt, kind="ExternalInput").ap()

    words = din("words", [NTILES, 128, 1], i32)
    emb = din("emb", [V, DPAD], bf16)
    ident = din("ident", [128, 128], bf16)
    # lhsT weights, gate-major free dim (slots i,f,g,o each 128 wide)
    wih0 = din("wih0", [2, 3, 128, 512], bf16)     # [dir][kchunk][K][4*128]
    wih1 = din("wih1", [2, 2, 128, 512], bf16)     # [dir][h0-dir kchunk][K][4*128]
    whh = din("whh", [2, 2, 128, 512], bf16)       # [layer][dir][K=H][4*128]
    biases = din("biases", [2, 2, 128, 4], f32)    # [layer][dir][hidden][gate]
    woutT = din("woutT", [2, 128, L], bf16)        # [h1-dir kchunk][K][L]
    bout = din("bout", [L, 1], f32)                # b_out - CBAR
    startv = din("startv", [L, 1], f32)
    expEb = din("expEb", [L, L], bf16)             # exp(trans), matmul lhsT
    expET = din("expET", [L, L], f32)              # exp(trans)^T
    expend = din("expend", [L, 1], f32)            # exp(end_t)
    oh = din("oh", [L, NT], bf16)                  # label one-hot, (t,b) order
    res = nc.dram_tensor("res", [1, 2], f32, kind="ExternalOutput").ap()

    with tile.TileContext(nc) as tc:
        _emit(tc, nc, Tsteps, NT, NTILES,
              words, emb, ident, wih0, wih1, whh, biases, woutT, bout,
              startv, expEb, expET, expend, oh, res)
    nc.compile()
    return nc


def _emit(tc, nc, Tsteps, NT, NTILES,
          words, emb, ident, wih0, wih1, whh, biases, woutT, bout,
          startv, expEb, expET, expend, oh, res):
    from contextlib import ExitStack

    NCH = max(1, Tsteps // 32)         # bulk chunks
    TCH = Tsteps // NCH                # time steps per bulk chunk
    CCOLS = TCH * BL                   # xp columns per bulk chunk
    chunks = _crf_chunks(Tsteps)
    NCRF = len(chunks)

    ctx = ExitStack()
    with ctx:
        consts = ctx.enter_context(tc.tile_pool(name="consts", bufs=1))
        states = ctx.enter_context(tc.tile_pool(name="states", bufs=1))

        # ---- persistent SBUF tiles ----
        ident_sb = consts.tile([128, 128], bf16, tag="ident")
        nc.sync.dma_start(ident_sb[:], ident[:])
        whh_sb = {}
        for l in range(2):
            for d in range(2):
                t_ = consts.tile([128, 512], bf16, name=f"whh{l}{d}")
                nc.sync.dma_start(t_[:], whh[l, d])
                whh_sb[l, d] = t_
        bias_sb = {}
        for l in range(2):
            for d in range(2):
                t_ = consts.tile([128, 4], f32, name=f"bias{l}{d}")
                nc.sync.dma_start(t_[:], biases[l, d])
                bias_sb[l, d] = t_
        wih0_sb = {}
        for d in range(2):
            for c in range(3):
                t_ = consts.tile([128, 512], bf16, name=f"wih0_{d}{c}")
                nc.sync.dma_start(t_[:], wih0[d, c])
                wih0_sb[d, c] = t_
        wih1_sb = {}
        for d in range(2):
            for k in range(2):
                t_ = consts.tile([128, 512], bf16, name=f"wih1_{d}{k}")
                nc.sync.dma_start(t_[:], wih1[d, k])
                wih1_sb[d, k] = t_
        woutT_sb = [consts.tile([128, L], bf16, name=f"wo{k}") for k in range(2)]
        for k in range(2):
            nc.sync.dma_start(woutT_sb[k][:], woutT[k])
        bout_sb = consts.tile([L, 1], f32, name="bout_sb")
        start_sb = consts.tile([L, 1], f32, name="start_sb")
        expEb_sb = consts.tile([L, L], bf16, name="expEb_sb")
        expET_sb = consts.tile([L, L], f32, name="expET_sb")
        expend_sb = consts.tile([L, 1], f32, name="expend_sb")
        nc.sync.dma_start(bout_sb[:], bout[:])
        nc.sync.dma_start(start_sb[:], startv[:])
        nc.sync.dma_start(expEb_sb[:], expEb[:])
        nc.sync.dma_start(expET_sb[:], expET[:])
        nc.sync.dma_start(expend_sb[:], expend[:])
        ones91 = consts.tile([L, 1], f32, name="ones91")
        ones91b = consts.tile([L, 1], bf16, name="ones91b")
        ones19 = consts.tile([1, L], f32, name="ones19")
        ones98 = consts.tile([L, BL], f32, name="ones98")
        nc.vector.memset(ones91[:], 1.0)
        nc.vector.memset(ones91b[:], 1.0)
        nc.vector.memset(ones19[:], 1.0)
        nc.vector.memset(ones98[:], 1.0)

        h_hist = {}
        for l in range(2):
            for d in range(2):
                h_hist[l, d] = states.tile([128, NT], bf16, name=f"h{l}{d}")
        C_st = {}
        for l in range(2):
            for d in range(2):
                C_st[l, d] = states.tile([128, BL], f32, name=f"C{l}{d}")

        # xp for the two directions of the current layer (reused across layers)
        xp_sb = [states.tile([128, 4 * NT], bf16, name=f"xp{d}") for d in range(2)]

        # emissions / CRF state
        em_sb = states.tile([L, NT], bf16, name="em")
        expem = states.tile([L, NT], f32, name="expem")
        oh_sb = states.tile([L, NT], bf16, name="oh_sb")
        nc.sync.dma_start(oh_sb[:], oh[:])
        num_parts = states.tile([L, NCRF], f32, name="num_parts")
        lnacc = states.tile([1, BL], f32, name="lnacc")
        r_sb = states.tile([L, BL], bf16, name="r_sb")
        nc.vector.memset(lnacc[:], 0.0)
        # X chunk-product tiles, j-major cols (col = j*BL + b)
        X_sb = [states.tile([L, L * BL], bf16, name=f"X{ci}")
                for ci in range(NCRF)]

        # =================================================================
        # embedding gather: all tiles up-front, interleaved from both ends
        # so early-needed chunks land first. DMA queues stream ahead of
        # compute.
        # =================================================================
        x_T = [states.tile([k, NT], bf16, name=f"xT{c}") for c, k in enumerate(KCH)]
        gorder = []
        lo, hi = 0, NTILES - 1
        while lo <= hi:
            gorder.append(lo)
            if hi != lo:
                gorder.append(hi)
            lo += 1
            hi -= 1
        gp_pool = ctx.enter_context(tc.tile_pool(name="gath", bufs=4))
        ip_pool = ctx.enter_context(tc.tile_pool(name="idx", bufs=4))
        for i in gorder:
            idx = ip_pool.tile([128, 1], i32, tag="idx", name="idx")
            nc.sync.dma_start(idx[:], words[i])
            g = gp_pool.tile([128, DPAD], bf16, tag="g", name="g")
            nc.gpsimd.indirect_dma_start(
                out=g[:], out_offset=None, in_=emb[:],
                in_offset=bass.IndirectOffsetOnAxis(ap=idx[:, :1], axis=0),
            )
            off = 0
            for c, k in enumerate(KCH):
                nc.sync.dma_start_transpose(
                    x_T[c][:, i * 128:(i + 1) * 128], g[:, off:off + k])
                off += k

        # =================================================================
        # bulk-matmul thunks (layer-0 input projection, layer-1 input
        # projection). One thunk = one (dir, gate-slot) chunk: K-accumulated
        # matmuls into PSUM + bias add into xp.
        # =================================================================
        b0pt = ctx.enter_context(tc.tile_pool(name="b0pt", bufs=2, space="PSUM"))

        def bulk_thunk(layer, d, slot, ch, pool):
            srcs = ([(x_T[c], wih0_sb[d, c], KCH[c]) for c in range(3)]
                    if layer == 0 else
                    [(h_hist[0, k], wih1_sb[d, k], 128) for k in range(2)])
            nsl = slice(ch * CCOLS, (ch + 1) * CCOLS)

            def fn():
                pt = pool.tile([128, CCOLS], f32, tag="pt", name="pt")
                for ki, (src, wt, kk) in enumerate(srcs):
                    nc.tensor.matmul(
                        pt[:],
                        lhsT=wt[:kk, slot * 128:(slot + 1) * 128],
                        rhs=src[:kk, nsl],
                        start=(ki == 0), stop=(ki == len(srcs) - 1),
                    )
                xv = xp_sb[d][:].rearrange("p (t g b) -> p t g b", g=4, b=BL)
                nc.vector.tensor_scalar(
                    out=xv[:, ch * TCH:(ch + 1) * TCH, slot, :],
                    in0=pt[:].rearrange("p (t b) -> p t b", b=BL),
                    scalar1=bias_sb[layer, d][:, slot:slot + 1],
                    scalar2=None, op0=ALU.add,
                )
            return fn

        def bulk_chunk_thunks(layer, ch, pool):
            return [bulk_thunk(layer, d, slot, ch, pool)
                    for d in range(2) for slot in range(4)]

        # =================================================================
        # CRF chunk thunks: emissions matmul + bias, exp, numerator
        # reduce, chunk-product init + steps.
        # =================================================================
        empt = ctx.enter_context(tc.tile_pool(name="empt", bufs=1, space="PSUM"))
        qtmp = ctx.enter_context(tc.tile_pool(name="qtmp", bufs=2, space="PSUM"))
        scrp = ctx.enter_context(tc.tile_pool(name="scrp", bufs=2))

        def crf_chunk_thunks(ci, t0, t1):
            c0 = (t0 - 1) * BL if t0 == 1 else t0 * BL   # chunk 0 also covers t=0
            c1 = t1 * BL
            X = X_sb[ci]
            Xr3 = X[:, 0:L * BL].rearrange("p (j b) -> p j b", b=BL)
            thunks = []

            w = c1 - c0
            hold_em = {}

            def em_mms():
                pt = empt.tile([L, CCOLS], f32, tag="empt", name="pt")
                for k in range(2):
                    nc.tensor.matmul(pt[:, 0:w], lhsT=woutT_sb[k][:],
                                     rhs=h_hist[1, k][:, c0:c1],
                                     start=(k == 0), stop=(k == 1))
                hold_em["pt"] = pt
            thunks.append(em_mms)

            def em_bias():
                nc.vector.tensor_scalar(
                    out=em_sb[:, c0:c1], in0=hold_em["pt"][:, 0:w],
                    scalar1=bout_sb[:, 0:1], scalar2=None, op0=ALU.add)
            thunks.append(em_bias)

            def em_exp():
                nc.scalar.activation(expem[:, c0:c1], em_sb[:, c0:c1], ACTF.Exp)
            thunks.append(em_exp)

            def em_num():
                scr = scrp.tile([L, c1 - c0], f32, tag=f"scr{c1 - c0}", name="scr")
                nc.vector.tensor_tensor(out=scr[:], in0=em_sb[:, c0:c1],
                                        in1=oh_sb[:, c0:c1], op=ALU.mult)
                nc.vector.tensor_reduce(num_parts[:, ci:ci + 1], scr[:],
                                        axis=mybir.AxisListType.X, op=ALU.add)
            thunks.append(em_num)

            def x_init():
                emB = expem[:, t0 * BL:(t0 + 1) * BL].unsqueeze(1) \
                    .broadcast_to([L, L, BL])
                ETB = expET_sb[:].unsqueeze(2).broadcast_to([L, L, BL])
                nc.vector.tensor_tensor(out=Xr3, in0=ETB, in1=emB, op=ALU.mult)
            thunks.append(x_init)

            for t in range(t0 + 1, t1):
                hold = {}

                def q_mm(t=t, hold=hold):
                    tmp = qtmp.tile([L, L * BL], f32, tag="tmp", name="tmp")
                    nc.tensor.matmul(tmp[:], lhsT=expEb_sb[:], rhs=X[:, 0:L * BL],
                                     start=True, stop=True)
                    hold["tmp"] = tmp

                def q_tt(t=t, hold=hold):
                    emB = expem[:, t * BL:(t + 1) * BL].unsqueeze(1) \
                        .broadcast_to([L, L, BL])
                    nc.vector.tensor_tensor(
                        out=Xr3,
                        in0=hold["tmp"][:].rearrange("p (j b) -> p j b", b=BL),
                        in1=emB, op=ALU.mult)
                thunks.append(q_mm)
                thunks.append(q_tt)
            return thunks

        # =================================================================
        # LSTM recurrence with side-work calendar hooks
        # =================================================================
        def recur_phase(layer, cal_pre, cal_post):
            for d in range(2):
                nc.vector.memset(C_st[layer, d][:], 0.0)
            with tc.tile_pool(name=f"gates{layer}", bufs=1, space="PSUM") as gpp, \
                 tc.tile_pool(name=f"y{layer}", bufs=4) as yp, \
                 tc.tile_pool(name=f"v{layer}", bufs=6) as vp:
                # one PSUM bank per direction, ring of 8 slot positions
                gring = [gpp.tile([128, 512], f32, name=f"gr{layer}{d}")
                         for d in range(2)]
                for t in range(Tsteps):
                    for fn in cal_pre.get(t, ()):
                        fn()
                    taus = (t, Tsteps - 1 - t)
                    first = (t == 0)
                    ys, tcs = {}, {}
                    rr = (t % 8) * 4 * BL
                    gps = {d: gring[d][:, rr:rr + 4 * BL] for d in range(2)}
                    for d in range(2):
                        tau = taus[d]
                        gp = gps[d]
                        nc.tensor.matmul(gp, lhsT=ident_sb[:],
                                         rhs=xp_sb[d][:, tau * 4 * BL:(tau + 1) * 4 * BL],
                                         start=True, stop=first)
                        if not first:
                            prev = tau - 1 if d == 0 else tau + 1
                            hh = h_hist[layer, d]
                            whh_t = whh_sb[layer, d]
                            for slot in range(4):
                                nc.tensor.matmul(
                                    gp[:, slot * BL:(slot + 1) * BL],
                                    lhsT=whh_t[:, slot * 128:(slot + 1) * 128],
                                    rhs=hh[:, prev * BL:(prev + 1) * BL],
                                    start=False, stop=(slot == 3))
                    for d in range(2):
                        y = yp.tile([128, 4 * BL], f32, tag=f"y{d}", name=f"y{d}")
                        ys[d] = y
                        nc.scalar.activation(y[:], gps[d], ACTF.Tanh, scale=0.5)
                    for d in range(2):
                        y, C = ys[d], C_st[layer, d]
                        v2 = vp.tile([128, BL], f32, tag=f"v2{d}", name=f"v2{d}")
                        z = vp.tile([128, BL], f32, tag=f"z{d}", name=f"z{d}")
                        nc.vector.scalar_tensor_tensor(
                            v2[:], in0=y[:, 0:BL], scalar=1.0, in1=y[:, 2 * BL:3 * BL],
                            op0=ALU.add, op1=ALU.mult)
                        nc.vector.scalar_tensor_tensor(
                            z[:], in0=y[:, BL:2 * BL], scalar=1.0, in1=C[:],
                            op0=ALU.add, op1=ALU.mult)
                        nc.vector.scalar_tensor_tensor(
                            C[:], in0=z[:], scalar=0.5, in1=v2[:],
                            op0=ALU.mult, op1=ALU.add)
                    for d in range(2):
                        tcl = vp.tile([128, BL], f32, tag=f"tc{d}", name=f"tc{d}")
                        tcs[d] = tcl
                        nc.scalar.activation(tcl[:], C_st[layer, d][:], ACTF.Tanh,
                                             scale=0.5)
                    for d in range(2):
                        tau = taus[d]
                        nc.vector.scalar_tensor_tensor(
                            h_hist[layer, d][:, tau * BL:(tau + 1) * BL],
                            in0=ys[d][:, 3 * BL:4 * BL], scalar=1.0, in1=tcs[d][:],
                            op0=ALU.add, op1=ALU.mult)
                    for fn in cal_post.get(t, ()):
                        fn()

        # calendar helper: place thunks one per half-slot (cap per half)
        def place(cal_pre, cal_post, thunks, start, cap=1):
            leftover = []
            s, h, c = start, 0, 0
            for fn in thunks:
                if s >= Tsteps:
                    leftover.append(fn)
                    continue
                (cal_pre if h == 0 else cal_post).setdefault(s, []).append(fn)
                c += 1
                if c >= cap:
                    c = 0
                    h += 1
                    if h == 2:
                        h = 0
                        s += 1
            return leftover

        # ---- L0 calendar: bulk0 chunk pairs streamed from both ends;
        #      bulk1 chunk pairs as soon as both dirs' h are final.
        cal0_pre, cal0_post = {}, {}
        prologue, gap_thunks = [], []
        if NCH >= 2:
            prologue += bulk_chunk_thunks(0, 0, b0pt)
            prologue += bulk_chunk_thunks(0, NCH - 1, b0pt)
            for j in range(1, (NCH + 1) // 2):
                th = bulk_chunk_thunks(0, j, b0pt)
                if NCH - 1 - j != j:
                    th += bulk_chunk_thunks(0, NCH - 1 - j, b0pt)
                place(cal0_pre, cal0_post, th, TCH * (j - 1))
            for c in range(1, NCH // 2):
                th = bulk_chunk_thunks(1, c, b0pt)
                if NCH - 1 - c != c:
                    th += bulk_chunk_thunks(1, NCH - 1 - c, b0pt)
                # h0b[TCH*c] lands during slot (Tsteps-1) - TCH*c; emit after
                place(cal0_pre, cal0_post, th, Tsteps - TCH * c)
            gap_thunks += bulk_chunk_thunks(1, 0, b0pt)
            gap_thunks += bulk_chunk_thunks(1, NCH - 1, b0pt)
            if NCH >= 2 and NCH // 2 <= NCH - 1 - (NCH // 2):
                # middle chunk (odd NCH) or none
                for c in range(NCH // 2, NCH - 1 - (NCH // 2) + 1):
                    place(cal0_pre, cal0_post, bulk_chunk_thunks(1, c, b0pt),
                          Tsteps - TCH * min(c, NCH - 1 - c))
        else:
            prologue += bulk_chunk_thunks(0, 0, b0pt)
            gap_thunks += bulk_chunk_thunks(1, 0, b0pt)

        # ---- L1 calendar: emissions + CRF chunk products.
        cal1_pre, cal1_post = {}, {}
        tails = []
        for ci, (t0, t1) in enumerate(chunks):
            tlo = t0 - 1 if t0 == 1 else t0   # em range also covers t=0
            a = max(t1 - 1, (Tsteps - 1) - tlo) + 1
            th = crf_chunk_thunks(ci, t0, t1)
            cap = 2 if a >= Tsteps - 72 else 1
            if a >= Tsteps:
                tails.append(th)
            else:
                left = place(cal1_pre, cal1_post, th, a, cap=cap)
                if left:
                    tails.append(left)

        # =================================================================
        # run it
        # =================================================================
        for fn in prologue:
            fn()
        recur_phase(0, cal0_pre, cal0_post)
        for fn in gap_thunks:
            fn()
        recur_phase(1, cal1_pre, cal1_post)

        # tail: leftover CRF chunks, round-robin interleaved for overlap
        maxlen = max((len(t) for t in tails), default=0)
        for k in range(maxlen):
            for tl in tails:
                if k < len(tl):
                    tl[k]()

        # =================================================================
        # CRF combine: r <- normalize(X_c^T r), descending over chunks.
        # Row L of each combine matmul output = sum of the incoming r
        # (ones column readout) used for the periodic renormalization.
        # =================================================================
        with tc.tile_pool(name="cmb", bufs=3, space="PSUM") as cmb, \
             tc.tile_pool(name="cmbs", bufs=4) as cmbs:
            # r init = expend (per-partition scalar mult of ones)
            nc.vector.tensor_scalar(out=r_sb[:], in0=ones98[:],
                                    scalar1=expend_sb[:, 0:1], scalar2=None,
                                    op0=ALU.mult)
            order = sorted(range(NCRF), key=lambda ci: -chunks[ci][0])
            for k, ci in enumerate(order):
                X = X_sb[ci]
                Xc = X[:].rearrange("p (j b) -> p b j", b=BL)  # [L, BL, L]
                renorm = (k % 3 == 2)
                if renorm:
                    # s = sum(r) before the update; lnacc += ln s; r /= s
                    sps = cmb.tile([L, BL], f32, tag="cps", name="sps")
                    nc.tensor.matmul(sps[0:1, :], lhsT=ones91b[:], rhs=r_sb[:],
                                     start=True, stop=True)
                rps = cmb.tile([L, BL], f32, tag="cps", name="rps")
                for b in range(BL):
                    nc.tensor.matmul(rps[:, b:b + 1], lhsT=Xc[:, b],
                                     rhs=r_sb[:, b:b + 1], start=True, stop=True)
                if renorm:
                    lns = cmbs.tile([1, BL], f32, tag="lns", name="lns")
                    nc.scalar.activation(lns[:], sps[0:1, :], ACTF.Ln)
                    nc.vector.tensor_tensor(out=lnacc[:], in0=lnacc[:],
                                            in1=lns[:], op=ALU.add)
                    rec = cmbs.tile([1, BL], f32, tag="rec", name="rec")
                    nc.vector.reciprocal(rec[:], sps[0:1, :])
                    rbt = cmb.tile([L, BL], f32, tag="cps", name="rbt")
                    nc.tensor.matmul(rbt[:], lhsT=ones19[:], rhs=rec[:],
                                     start=True, stop=True)
                    rtmp = cmbs.tile([L, BL], f32, tag="rtmp", name="rtmp")
                    nc.vector.tensor_scalar(out=rtmp[:], in0=rps[:],
                                            scalar1=1.0, scalar2=None,
                                            op0=ALU.mult)
                    nc.vector.tensor_tensor(out=r_sb[:], in0=rtmp[:],
                                            in1=rbt[:], op=ALU.mult)
                else:
                    nc.vector.tensor_scalar(out=r_sb[:], in0=rps[:],
                                            scalar1=1.0, scalar2=None,
                                            op0=ALU.mult)

            # finalize: P0 = exp(em_0 + start); z = r . P0; denom = ln z + lnacc
            P0 = cmbs.tile([L, BL], f32, tag="P0", name="P0")
            nc.scalar.activation(P0[:], em_sb[:, 0:BL], ACTF.Exp,
                                 bias=start_sb[:, 0:1])
            prod = cmbs.tile([L, BL], f32, tag="prod", name="prod")
            nc.vector.tensor_tensor(out=prod[:], in0=P0[:], in1=r_sb[:],
                                    op=ALU.mult)
            zps = cmb.tile([L, BL], f32, tag="cps", name="zps")
            nc.tensor.matmul(zps[0:1, :], lhsT=ones91[:], rhs=prod[:],
                             start=True, stop=True)
            lnz = cmbs.tile([1, BL], f32, tag="lnz", name="lnz")
            nc.scalar.activation(lnz[:], zps[0:1, :], ACTF.Ln)
            nc.vector.tensor_tensor(out=lnz[:], in0=lnz[:], in1=lnacc[:],
                                    op=ALU.add)
            dsc = cmbs.tile([1, 1], f32, tag="dsc", name="dsc")
            nc.vector.tensor_reduce(dsc[:], lnz[:], axis=mybir.AxisListType.X,
                                    op=ALU.add)
            num9 = cmbs.tile([L, 1], f32, tag="num9", name="num9")
            nc.vector.tensor_reduce(num9[:], num_parts[:],
                                    axis=mybir.AxisListType.X, op=ALU.add)
            nps = cmb.tile([L, BL], f32, tag="cps", name="nps")
            nc.tensor.matmul(nps[0:1, 0:1], lhsT=ones91[:], rhs=num9[:, 0:1],
                             start=True, stop=True)
            out_sb = cmbs.tile([1, 2], f32, tag="out_sb", name="out_sb")
            nc.vector.tensor_scalar(out=out_sb[:, 0:1], in0=nps[0:1, 0:1],
                                    scalar1=0.0, scalar2=None, op0=ALU.add)
            nc.vector.tensor_scalar(out=out_sb[:, 1:2], in0=dsc[:],
                                    scalar1=0.0, scalar2=None, op0=ALU.add)
            nc.sync.dma_start(res[:], out_sb[:])


# ---------------------------------------------------------------------------
# host side
# ---------------------------------------------------------------------------

def _prescale(w_ih, w_hh, b_ih, b_hh, h_in_doubled):
    """Gate-slot layout is torch order (i,f,g,o). Returns fp32 arrays."""
    sg = np.ones((4, 1), np.float32)
    sg[2] = 2.0                       # g gate rows x2 (tanh scale 0.5 trick)
    srows = np.repeat(sg, H, axis=0)  # [512, 1]
    wih = w_ih.astype(np.float32) * srows
    whh_ = w_hh.astype(np.float32) * srows * 0.5
    b = (b_ih + b_hh).astype(np.float32) * srows[:, 0]
    if h_in_doubled:
        wih = wih * 0.5
    return wih, whh_, b


def _lhsT_gate_major(w, kchunks):
    """w: [4H, K] fp32 -> [nchunk, 128, 512] bf16 lhsT (zero-padded K)."""
    outs = []
    off = 0
    for kk in kchunks:
        blk = np.zeros((128, 512), np.float32)
        take = min(kk, w.shape[1] - off)
        for slot in range(4):
            blk[:take, slot * 128:(slot + 1) * 128] = \
                w[slot * H:(slot + 1) * H, off:off + take].T
        outs.append(blk)
        off += kk
    return np.stack(outs).astype(np.dtype("bfloat16"))


_PROG_CACHE = {}


def _get_program(Tsteps):
    if Tsteps not in _PROG_CACHE:
        _PROG_CACHE[Tsteps] = build_program(Tsteps)
    return _PROG_CACHE[Tsteps]


def prepare_inputs(inputs, Tsteps=T):
    """Build the per-core input maps + the host numerator constants."""
    bfl = np.dtype("bfloat16")
    words = np.asarray(inputs["word_batch"]).astype(np.int64)
    labels = np.asarray(inputs["label_batch"]).astype(np.int64)
    emb = np.asarray(inputs["emb"], np.float32)
    words = words[:, :Tsteps]
    labels = labels[:, :Tsteps]

    embp = np.zeros((V, DPAD), np.float32)
    embp[:, :D] = emb
    embp = embp.astype(bfl)

    ident = np.eye(128, dtype=np.float32).astype(bfl)

    wih0_l, whh_l, wih1_l, bias_l = [], [], [], []
    for layer, (wihk, whhk, bihk, bhhk) in enumerate(
            [("w_ih_l0", "w_hh_l0", "b_ih_l0", "b_hh_l0"),
             ("w_ih_l1", "w_hh_l1", "b_ih_l1", "b_hh_l1")]):
        for d in range(2):
            wih, whh_, b = _prescale(
                np.asarray(inputs[wihk])[d], np.asarray(inputs[whhk])[d],
                np.asarray(inputs[bihk])[d], np.asarray(inputs[bhhk])[d],
                h_in_doubled=(layer == 1))
            if layer == 0:
                wihp = np.zeros((512, DPAD), np.float32)
                wihp[:, :D] = wih
                wih0_l.append(_lhsT_gate_major(wihp, KCH))
            else:
                wih1_l.append(_lhsT_gate_major(wih, (128, 128)))
            whh_l.append(_lhsT_gate_major(whh_, (128,)))
            bias_l.append(b.reshape(4, H).T)  # [128, 4] gate-major cols
    wih0 = np.stack(wih0_l)                       # [2, 3, 128, 512]
    wih1 = np.stack(wih1_l)                       # [2, 2, 128, 512]
    whh = np.stack(whh_l).reshape(2, 2, 1, 128, 512)[:, :, 0]
    biases = np.stack(bias_l).reshape(2, 2, 128, 4).astype(np.float32)

    w_out = np.asarray(inputs["w_out"], np.float32) * 0.5   # [L, 2H]
    woutT = np.stack([w_out[:, :H].T, w_out[:, H:].T]).astype(bfl)  # [2,128,L]
    bout = (np.asarray(inputs["b_out"], np.float32) - CBAR).reshape(L, 1)
    startv = np.asarray(inputs["start_t"], np.float32).reshape(L, 1)
    expE = np.exp(np.asarray(inputs["trans"], np.float32))
    expEb = expE.astype(bfl)
    expET = expE.T.copy().astype(np.float32)
    expend = np.exp(np.asarray(inputs["end_t"], np.float32)).reshape(L, 1)

    NT = Tsteps * BL
    in_maps = []
    num_consts = []
    start_t = np.asarray(inputs["start_t"], np.float32)
    end_t = np.asarray(inputs["end_t"], np.float32)
    trans = np.asarray(inputs["trans"], np.float32)
    for c in range(NCORES):
        bs = slice(c * BL, (c + 1) * BL)
        wc = words[bs]                        # [BL, Tsteps]
        lc = labels[bs]
        toks = wc.T.reshape(-1).astype(np.int32)          # (t, b) order
        ohc = (lc.T.reshape(1, -1) == np.arange(L).reshape(L, 1))
        in_maps.append({
            "words": toks.reshape(NT // 128, 128, 1),
            "emb": embp, "ident": ident,
            "wih0": wih0, "wih1": wih1, "whh": whh, "biases": biases,
            "woutT": woutT, "bout": bout, "startv": startv,
            "expEb": expEb, "expET": expET, "expend": expend,
            "oh": ohc.astype(bfl),
        })
        num_consts.append(
            float(start_t[lc[:, 0]].sum())
            + float(trans[lc[:, :-1], lc[:, 1:]].sum())
            + float(end_t[lc[:, -1]].sum()))
    return in_maps, num_consts


def kernel(**inputs):
    in_maps, num_consts = prepare_inputs(inputs, T)
    nc = _get_program(T)
    out = run_bass_kernel_spmd(nc, in_maps, list(range(NCORES)))
    llh = 0.0
    for c in range(NCORES):
        r = out.results[c]["res"].reshape(2).astype(np.float64)
        llh += num_consts[c] + r[0] - r[1]
    return np.float32(-llh)


if __name__ == "__main__":
    np.random.seed(0)
    print("building program ...")
    build_program(T)
    print("ok")
